# revision 32
# baseline (speedup 1.0000x reference)
"""Trainium2 Bass kernel for a causal-EMA encoder:

    out = EMA3(x @ W_down^T) @ W_up^T

with EMA layer i:  y_t = a_i * y_{t-1} + (1 - a_i) * h_t,  a_i = sigmoid(log_a[i]).

Shapes (hardcoded): x [4, 4096, 2048], W_down [512, 2048], W_up [2048, 512],
log_a [3, 512]. Output [4, 4096, 2048] fp32.

Strategy (8 NeuronCores, SPMD, no collectives):
  * Shard (batch, sequence-half): core c handles batch c//2, L-half c%2.
  * The EMA scans are causal with decay a ~ sigmoid(3) ≈ 0.95, so state
    contributions die off geometrically. Second-half cores recompute a
    KWARM-token "warmup" prefix instead of communicating carry state; the
    first-half cores get a zero-padded warmup so all cores run one program.
  * Linearity: scan_i((1-a_i) v) = (1-a_i) scan_i(v), so the three input
    injections fold into ONE per-channel pre-scale prod_i(1-a_i), then three
    pure a-decay scans, which map 1:1 onto the DVE TensorTensorScan ISA op.
  * All matmuls are fp16 (same PE throughput as bf16, 3 more mantissa bits),
    accumulating fp32 in PSUM. Scan state/carry stays fp32.
  * Transpose-free device code: the host feeds x already transposed per core
    as xT [D, LC] fp16 and receives outT [D, 2048] fp32, so the contraction
    dim is on partitions for every matmul and every DMA is wide-contiguous.
"""

import sys

for _p in ("/opt/trn_rl_repo", "/root/.axon_site/_ro/trn_rl_repo"):
    if _p not in sys.path:
        sys.path.append(_p)

import numpy as np
from contextlib import ExitStack

import concourse.tile as tile
from concourse import bacc, mybir
from concourse.bass_utils import run_bass_kernel_spmd

B, L, D, DI, NL = 4, 4096, 2048, 512, 3
P = 128
N_CORES = 8
HALF = L // 2          # tokens produced per core
CHUNK = 512            # l-chunk (= max fp32 PSUM free dim)
NKD = D // P           # 16 k-tiles for down-proj
NME = DI // P          # 4  e-tiles (down-proj m / up-proj k)
NMD = D // P           # 16 dd-tiles for up-proj

FP16 = mybir.dt.float16
F32 = mybir.dt.float32
MULT = mybir.AluOpType.mult
ADD = mybir.AluOpType.add

_module_cache: dict[int, object] = {}
LAST_RESULTS = None  # BassKernelResults of the most recent run (for profiling)


def _build_body(ctx: ExitStack, tc: tile.TileContext, kwarm: int):
    nc = tc.nc
    lc = HALF + kwarm
    # chunk widths: warmup chunks first (a single short chunk when
    # kwarm <= 512), then HALF//CHUNK full output chunks
    if kwarm <= CHUNK:
        warm_widths = [kwarm] if kwarm else []
    else:
        assert kwarm % CHUNK == 0
        warm_widths = [CHUNK] * (kwarm // CHUNK)
    widths = warm_widths + [CHUNK] * (HALF // CHUNK)
    warm_chunks = len(warm_widths)
    nchunk = len(widths)

    xT = nc.dram_tensor("xT", [D, lc], FP16, kind="ExternalInput").ap()
    wdT = nc.dram_tensor("wdT", [D, DI], FP16, kind="ExternalInput").ap()
    wuT = nc.dram_tensor("wuT", [DI, D], FP16, kind="ExternalInput").ap()
    # decay: a per (e-tile, channel, layer); scale: prod_i(1-a_i) per (e-tile, channel)
    decay = nc.dram_tensor("decay", [NME, P, NL], F32, kind="ExternalInput").ap()
    scale = nc.dram_tensor("scale", [NME, P, 1], F32, kind="ExternalInput").ap()
    outT = nc.dram_tensor("outT", [D, HALF], F32, kind="ExternalOutput").ap()

    singles = ctx.enter_context(tc.tile_pool(name="singles", bufs=1))
    xpool = ctx.enter_context(tc.tile_pool(name="xpool", bufs=3))
    hpool = ctx.enter_context(tc.tile_pool(name="hpool", bufs=4))
    zpool = ctx.enter_context(tc.tile_pool(name="zpool", bufs=4))
    zhpool = ctx.enter_context(tc.tile_pool(name="zhpool", bufs=8))
    opool = ctx.enter_context(tc.tile_pool(name="opool", bufs=8))
    psum_h = ctx.enter_context(tc.tile_pool(name="psum_h", bufs=2, space="PSUM"))
    psum_o = ctx.enter_context(tc.tile_pool(name="psum_o", bufs=6, space="PSUM"))

    # ---- persistent weights / per-channel constants ----
    # DMAs for these are emitted inside the chunk loop: down-proj weight
    # pieces interleave with the first x chunk so PE can start after ~1MB of
    # DMA instead of 6MB, and up-proj weights queue behind chunk 1's x.
    dec_sb = singles.tile([P, NME, NL], F32)
    sc_sb = singles.tile([P, NME, 1], F32)
    wd_sb = singles.tile([P, NKD, DI], FP16)
    wdTr = wdT.rearrange("(kt p) e -> p kt e", p=P)
    wu_sb = singles.tile([P, NME, D], FP16)

    # Per-(e-tile, layer) decay rows broadcast along the chunk (materialized
    # at j==0 below), since TensorTensorScan's data0 is a full [P, CHUNK]
    # tensor.
    ones = singles.tile([P, CHUNK], F32)
    nc.vector.memset(ones, 1.0)
    a_sb = singles.tile([P, NME, NL, CHUNK], F32)

    # Per-(e-tile, layer) scan carry state: last column of the previous
    # chunk's scan output. Separate tiny tiles so Tile's dependency tracking
    # serializes only the true per-(m, layer) carry chain.
    carry = [
        [
            singles.tile([P, 1], F32, tag=f"carry_{m}_{i}", name=f"carry_{m}_{i}")
            for i in range(NL)
        ]
        for m in range(NME)
    ]

    xTr = xT.rearrange("(kt p) l -> p kt l", p=P)
    outTr = outT.rearrange("(mt p) l -> p mt l", p=P)

    l0 = 0
    for j, w in enumerate(widths):
        x_sb = xpool.tile([P, NKD, CHUNK], FP16, tag="x")
        # 4-k-tile DMA pieces so the k-loop can start on piece 0; on chunk 0
        # interleave the down-proj weight pieces with the x pieces.
        for piece in range(0, NKD, 4):
            if j == 0:
                nc.sync.dma_start(
                    out=wd_sb[:, piece : piece + 4, :],
                    in_=wdTr[:, piece : piece + 4, :],
                )
            nc.sync.dma_start(
                out=x_sb[:, piece : piece + 4, :w],
                in_=xTr[:, piece : piece + 4, l0 : l0 + w],
            )
        if j == 0:
            # constants for the scans (needed ~6us in) load after the
            # critical path
            nc.sync.dma_start(out=dec_sb, in_=decay.rearrange("t p l -> p t l"))
            nc.sync.dma_start(out=sc_sb, in_=scale.rearrange("t p o -> p t o"))
            for t in range(NME):
                for i in range(NL):
                    nc.vector.tensor_scalar_mul(
                        a_sb[:, t, i, :], ones, dec_sb[:, t, i : i + 1]
                    )
        if j == min(1, nchunk - 1):
            # up-proj weights aren't needed until the first output chunk;
            # queue them behind chunk 1's x so that stream isn't delayed
            nc.sync.dma_start(out=wu_sb, in_=wuT.rearrange("(kt p) d -> p kt d", p=P))

        z3h = [None] * NME
        for m in range(NME):
            # ---- down-proj: h^T[e, l] = W_down^T.T @ x^T, contract over d ----
            ph = psum_h.tile([P, CHUNK], F32, tag="ph")
            for k in range(NKD):
                nc.tensor.matmul(
                    ph[:, :w],
                    lhsT=wd_sb[:, k, m * P : (m + 1) * P],
                    rhs=x_sb[:, k, :w],
                    start=(k == 0),
                    stop=(k == NKD - 1),
                )
            # evacuate PSUM (on ScalarE, keeping DVE free for the scans) with
            # the fused prod(1-a_i) input-injection scale
            hsc = hpool.tile([P, CHUNK], F32, tag="hsc")
            nc.scalar.mul(hsc[:, :w], ph[:, :w], sc_sb[:, m, 0:1])

            # ---- three chained EMA scans along the free (L) dim ----
            zin = hsc
            zlast = None
            for i in range(NL):
                zi = zpool.tile([P, CHUNK], F32, tag=f"z{i}")
                nc.vector.tensor_tensor_scan(
                    zi[:, :w], a_sb[:, m, i, :w], zin[:, :w],
                    initial=(0.0 if j == 0 else carry[m][i]),
                    op0=MULT, op1=ADD,
                )
                if j < nchunk - 1:
                    nc.vector.tensor_copy(out=carry[m][i], in_=zi[:, w - 1 : w])
                zin = zi
                zlast = zi

            if j >= warm_chunks:
                zh = zhpool.tile([P, CHUNK], FP16, tag="zh")
                nc.vector.tensor_copy(out=zh[:, :w], in_=zlast[:, :w])
                z3h[m] = zh

        if j >= warm_chunks:
            lo = l0 - kwarm
            # ---- up-proj: out^T[dd, l] = W_up^T.T @ y^T, contract over e ----
            for mm in range(NMD):
                po = psum_o.tile([P, CHUNK], F32, tag="po")
                for k in range(NME):
                    nc.tensor.matmul(
                        po[:, :w],
                        lhsT=wu_sb[:, k, mm * P : (mm + 1) * P],
                        rhs=z3h[k][:, :w],
                        start=(k == 0),
                        stop=(k == NME - 1),
                    )
                osb = opool.tile([P, CHUNK], F32, tag="osb")
                # split the final chunk's evacuations across ScalarE and DVE
                # so the kernel tail isn't serialized on one engine
                if j == nchunk - 1 and mm % 2 == 1:
                    nc.vector.tensor_copy(out=osb[:, :w], in_=po[:, :w])
                else:
                    nc.scalar.copy(out=osb[:, :w], in_=po[:, :w])
                nc.sync.dma_start(out=outTr[:, mm, lo : lo + w], in_=osb[:, :w])
        l0 += w


def _get_module(kwarm: int):
    if kwarm in _module_cache:
        return _module_cache[kwarm]
    nc = bacc.Bacc("TRN2", target_bir_lowering=False, debug=False, enable_asserts=False)
    with tile.TileContext(nc) as tc:
        with ExitStack() as ctx:
            _build_body(ctx, tc, kwarm)
    nc.compile()
    _module_cache[kwarm] = nc
    return nc


def _pick_kwarm(a: np.ndarray) -> int:
    """Smallest KWARM (multiple of 64, capped) such that truncating scan
    history to KWARM tokens perturbs outputs by ~1e-5 of the h scale (an
    order below the fp16 matmul noise floor). 3-layer composed impulse
    response: the lag-k weight is (1-a)^3 * C(k+2,2) * a^k."""
    a64 = a.astype(np.float64)

    def tail(k):
        return float(np.max(0.5 * (k + 2) * (k + 1) * (a64**k) * (1.0 - a64) ** 3))

    k = 128
    while k < 2048 and tail(k) >= 1e-5:
        k += 64 if k < CHUNK else CHUNK
    return k


def kernel(x, W_down, W_up, log_a):
    global LAST_RESULTS
    x = np.ascontiguousarray(np.asarray(x, dtype=np.float32))
    W_down = np.asarray(W_down, dtype=np.float32)
    W_up = np.asarray(W_up, dtype=np.float32)
    log_a = np.asarray(log_a, dtype=np.float32)
    assert x.shape == (B, L, D) and W_down.shape == (DI, D) and W_up.shape == (D, DI)

    a64 = 1.0 / (1.0 + np.exp(-log_a.astype(np.float64)))          # [NL, DI]
    a = a64.astype(np.float32)
    scale = np.prod(1.0 - a64, axis=0).astype(np.float32)          # [DI]

    kwarm = _pick_kwarm(a)
    lc = HALF + kwarm
    nc = _get_module(kwarm)

    wdT = np.ascontiguousarray(W_down.T).astype(np.float16)
    wuT = np.ascontiguousarray(W_up.T).astype(np.float16)
    decay = np.ascontiguousarray(a.T.reshape(NME, P, NL))          # [t, p, l]
    scale_r = np.ascontiguousarray(scale.reshape(NME, P, 1))

    in_maps = []
    for c in range(N_CORES):
        b, h = divmod(c, 2)
        xt = np.zeros((lc, D), dtype=np.float32)
        lstart = h * HALF - kwarm
        src_lo = max(0, lstart)
        xt[src_lo - lstart :, :] = x[b, src_lo : h * HALF + HALF, :]
        xT = np.ascontiguousarray(xt.T).astype(np.float16)          # [D, lc]
        in_maps.append(
            {"xT": xT, "wdT": wdT, "wuT": wuT, "decay": decay, "scale": scale_r}
        )

    res = run_bass_kernel_spmd(nc, in_maps, core_ids=list(range(N_CORES)))
    LAST_RESULTS = res

    out = np.empty((B, L, D), dtype=np.float32)
    for c in range(N_CORES):
        b, h = divmod(c, 2)
        out[b, h * HALF : (h + 1) * HALF, :] = res.results[c]["outT"].T
    return out


# revision 38
# speedup vs baseline: 1.0076x; 1.0076x over previous
"""Trainium2 Bass kernel for a causal-EMA encoder:

    out = EMA3(x @ W_down^T) @ W_up^T

with EMA layer i:  y_t = a_i * y_{t-1} + (1 - a_i) * h_t,  a_i = sigmoid(log_a[i]).

Shapes (hardcoded): x [4, 4096, 2048], W_down [512, 2048], W_up [2048, 512],
log_a [3, 512]. Output [4, 4096, 2048] fp32.

Strategy (8 NeuronCores, SPMD, no collectives):
  * Shard (batch, sequence-half): core c handles batch c//2, L-half c%2.
  * The EMA scans are causal with decay a ~ sigmoid(3) ≈ 0.95, so state
    contributions die off geometrically. Second-half cores recompute a
    KWARM-token "warmup" prefix instead of communicating carry state; the
    first-half cores get a zero-padded warmup so all cores run one program.
  * Linearity: scan_i((1-a_i) v) = (1-a_i) scan_i(v), so the three input
    injections fold into ONE per-channel pre-scale prod_i(1-a_i), then three
    pure a-decay scans, which map 1:1 onto the DVE TensorTensorScan ISA op.
  * All matmuls are fp16 (same PE throughput as bf16, 3 more mantissa bits),
    accumulating fp32 in PSUM. Scan state/carry stays fp32.
  * Transpose-free device code: the host feeds x already transposed per core
    as xT [D, LC] fp16 and receives outT [D, 2048] fp32, so the contraction
    dim is on partitions for every matmul and every DMA is wide-contiguous.
"""

import sys

for _p in ("/opt/trn_rl_repo", "/root/.axon_site/_ro/trn_rl_repo"):
    if _p not in sys.path:
        sys.path.append(_p)

import numpy as np
from contextlib import ExitStack

import concourse.tile as tile
from concourse import bacc, mybir
from concourse.bass_utils import run_bass_kernel_spmd

B, L, D, DI, NL = 4, 4096, 2048, 512, 3
P = 128
N_CORES = 8
HALF = L // 2          # tokens produced per core
CHUNK = 512            # l-chunk (= max fp32 PSUM free dim)
NKD = D // P           # 16 k-tiles for down-proj
NME = DI // P          # 4  e-tiles (down-proj m / up-proj k)
NMD = D // P           # 16 dd-tiles for up-proj

FP16 = mybir.dt.float16
F32 = mybir.dt.float32
MULT = mybir.AluOpType.mult
ADD = mybir.AluOpType.add

_module_cache: dict[int, object] = {}
LAST_RESULTS = None  # BassKernelResults of the most recent run (for profiling)


def _build_body(ctx: ExitStack, tc: tile.TileContext, kwarm: int):
    nc = tc.nc
    lc = HALF + kwarm
    # chunk widths: warmup chunks first (a single short chunk when
    # kwarm <= 512), then HALF//CHUNK full output chunks
    if kwarm <= CHUNK:
        warm_widths = [kwarm] if kwarm else []
    else:
        assert kwarm % CHUNK == 0
        warm_widths = [CHUNK] * (kwarm // CHUNK)
    widths = warm_widths + [CHUNK] * (HALF // CHUNK)
    warm_chunks = len(warm_widths)
    nchunk = len(widths)

    xT = nc.dram_tensor("xT", [D, lc], FP16, kind="ExternalInput").ap()
    wdT = nc.dram_tensor("wdT", [D, DI], FP16, kind="ExternalInput").ap()
    wuT = nc.dram_tensor("wuT", [DI, D], FP16, kind="ExternalInput").ap()
    # decay: a per (e-tile, channel, layer); scale: prod_i(1-a_i) per (e-tile, channel)
    decay = nc.dram_tensor("decay", [NME, P, NL], F32, kind="ExternalInput").ap()
    scale = nc.dram_tensor("scale", [NME, P, 1], F32, kind="ExternalInput").ap()
    outT = nc.dram_tensor("outT", [D, HALF], F32, kind="ExternalOutput").ap()

    singles = ctx.enter_context(tc.tile_pool(name="singles", bufs=1))
    xpool = ctx.enter_context(tc.tile_pool(name="xpool", bufs=3))
    hpool = ctx.enter_context(tc.tile_pool(name="hpool", bufs=4))
    zpool = ctx.enter_context(tc.tile_pool(name="zpool", bufs=4))
    zhpool = ctx.enter_context(tc.tile_pool(name="zhpool", bufs=8))
    opool = ctx.enter_context(tc.tile_pool(name="opool", bufs=8))
    psum_h = ctx.enter_context(tc.tile_pool(name="psum_h", bufs=2, space="PSUM"))
    psum_o = ctx.enter_context(tc.tile_pool(name="psum_o", bufs=6, space="PSUM"))

    # ---- persistent weights / per-channel constants ----
    # DMAs for these are emitted inside the chunk loop: down-proj weight
    # pieces interleave with the first x chunk so PE can start after ~1MB of
    # DMA instead of 6MB, and up-proj weights queue behind chunk 1's x.
    dec_sb = singles.tile([P, NME, NL], F32)
    sc_sb = singles.tile([P, NME, 1], F32)
    wd_sb = singles.tile([P, NKD, DI], FP16)
    wdTr = wdT.rearrange("(kt p) e -> p kt e", p=P)
    wu_sb = singles.tile([P, NME, D], FP16)

    # Per-(e-tile, layer) decay rows broadcast along the chunk (materialized
    # at j==0 below), since TensorTensorScan's data0 is a full [P, CHUNK]
    # tensor.
    ones = singles.tile([P, CHUNK], F32)
    nc.vector.memset(ones, 1.0)
    a_sb = singles.tile([P, NME, NL, CHUNK], F32)

    # Per-(e-tile, layer) scan carry state: last column of the previous
    # chunk's scan output. Separate tiny tiles so Tile's dependency tracking
    # serializes only the true per-(m, layer) carry chain.
    carry = [
        [
            singles.tile([P, 1], F32, tag=f"carry_{m}_{i}", name=f"carry_{m}_{i}")
            for i in range(NL)
        ]
        for m in range(NME)
    ]

    xTr = xT.rearrange("(kt p) l -> p kt l", p=P)
    outTr = outT.rearrange("(mt p) l -> p mt l", p=P)

    l0 = 0
    for j, w in enumerate(widths):
        x_sb = xpool.tile([P, NKD, CHUNK], FP16, tag="x")
        # k-tile DMA pieces so the k-loop can start on piece 0; on chunk 0
        # interleave the down-proj weight pieces with the x pieces, with
        # finer granularity up front so the first matmul starts sooner.
        pieces = [(0, 2), (2, 2), (4, 2), (6, 2), (8, 4), (12, 4)] if j == 0 else [
            (0, 4), (4, 4), (8, 4), (12, 4)
        ]
        for p0, sz in pieces:
            if j == 0:
                nc.sync.dma_start(
                    out=wd_sb[:, p0 : p0 + sz, :],
                    in_=wdTr[:, p0 : p0 + sz, :],
                )
            nc.sync.dma_start(
                out=x_sb[:, p0 : p0 + sz, :w],
                in_=xTr[:, p0 : p0 + sz, l0 : l0 + w],
            )
        if j == 0:
            # constants for the scans (needed ~6us in) load after the
            # critical path
            nc.sync.dma_start(out=dec_sb, in_=decay.rearrange("t p l -> p t l"))
            nc.sync.dma_start(out=sc_sb, in_=scale.rearrange("t p o -> p t o"))
            for t in range(NME):
                for i in range(NL):
                    nc.vector.tensor_scalar_mul(
                        a_sb[:, t, i, :], ones, dec_sb[:, t, i : i + 1]
                    )
        if j == min(1, nchunk - 1):
            # up-proj weights aren't needed until the first output chunk;
            # queue them behind chunk 1's x so that stream isn't delayed
            nc.sync.dma_start(out=wu_sb, in_=wuT.rearrange("(kt p) d -> p kt d", p=P))

        z3h = [None] * NME
        for m in range(NME):
            # ---- down-proj: h^T[e, l] = W_down^T.T @ x^T, contract over d ----
            ph = psum_h.tile([P, CHUNK], F32, tag="ph")
            for k in range(NKD):
                nc.tensor.matmul(
                    ph[:, :w],
                    lhsT=wd_sb[:, k, m * P : (m + 1) * P],
                    rhs=x_sb[:, k, :w],
                    start=(k == 0),
                    stop=(k == NKD - 1),
                )
            # evacuate PSUM (on ScalarE, keeping DVE free for the scans) with
            # the fused prod(1-a_i) input-injection scale
            hsc = hpool.tile([P, CHUNK], F32, tag="hsc")
            nc.scalar.mul(hsc[:, :w], ph[:, :w], sc_sb[:, m, 0:1])

            # ---- three chained EMA scans along the free (L) dim ----
            zin = hsc
            zlast = None
            for i in range(NL):
                zi = zpool.tile([P, CHUNK], F32, tag=f"z{i}")
                nc.vector.tensor_tensor_scan(
                    zi[:, :w], a_sb[:, m, i, :w], zin[:, :w],
                    initial=(0.0 if j == 0 else carry[m][i]),
                    op0=MULT, op1=ADD,
                )
                if j < nchunk - 1:
                    nc.vector.tensor_copy(out=carry[m][i], in_=zi[:, w - 1 : w])
                zin = zi
                zlast = zi

            if j >= warm_chunks:
                zh = zhpool.tile([P, CHUNK], FP16, tag="zh")
                nc.vector.tensor_copy(out=zh[:, :w], in_=zlast[:, :w])
                z3h[m] = zh

        if j >= warm_chunks:
            lo = l0 - kwarm
            # ---- up-proj: out^T[dd, l] = W_up^T.T @ y^T, contract over e ----
            for mm in range(NMD):
                po = psum_o.tile([P, CHUNK], F32, tag="po")
                for k in range(NME):
                    nc.tensor.matmul(
                        po[:, :w],
                        lhsT=wu_sb[:, k, mm * P : (mm + 1) * P],
                        rhs=z3h[k][:, :w],
                        start=(k == 0),
                        stop=(k == NME - 1),
                    )
                osb = opool.tile([P, CHUNK], F32, tag="osb")
                # alternate evacuations across ScalarE and DVE so neither
                # engine's queue paces the store stream or the kernel tail
                if mm % 2 == 1:
                    nc.vector.tensor_copy(out=osb[:, :w], in_=po[:, :w])
                else:
                    nc.scalar.copy(out=osb[:, :w], in_=po[:, :w])
                nc.sync.dma_start(out=outTr[:, mm, lo : lo + w], in_=osb[:, :w])
        l0 += w


def _get_module(kwarm: int):
    if kwarm in _module_cache:
        return _module_cache[kwarm]
    nc = bacc.Bacc("TRN2", target_bir_lowering=False, debug=False, enable_asserts=False)
    with tile.TileContext(nc) as tc:
        with ExitStack() as ctx:
            _build_body(ctx, tc, kwarm)
    nc.compile()
    _module_cache[kwarm] = nc
    return nc


def _pick_kwarm(a: np.ndarray) -> int:
    """Smallest KWARM (multiple of 64, capped) such that truncating scan
    history to KWARM tokens perturbs outputs by ~1e-5 of the h scale (an
    order below the fp16 matmul noise floor). 3-layer composed impulse
    response: the lag-k weight is (1-a)^3 * C(k+2,2) * a^k."""
    a64 = a.astype(np.float64)

    def tail(k):
        return float(np.max(0.5 * (k + 2) * (k + 1) * (a64**k) * (1.0 - a64) ** 3))

    k = 128
    while k < 2048 and tail(k) >= 2e-5:
        k += 64 if k < CHUNK else CHUNK
    return k


def kernel(x, W_down, W_up, log_a):
    global LAST_RESULTS
    x = np.ascontiguousarray(np.asarray(x, dtype=np.float32))
    W_down = np.asarray(W_down, dtype=np.float32)
    W_up = np.asarray(W_up, dtype=np.float32)
    log_a = np.asarray(log_a, dtype=np.float32)
    assert x.shape == (B, L, D) and W_down.shape == (DI, D) and W_up.shape == (D, DI)

    a64 = 1.0 / (1.0 + np.exp(-log_a.astype(np.float64)))          # [NL, DI]
    a = a64.astype(np.float32)
    scale = np.prod(1.0 - a64, axis=0).astype(np.float32)          # [DI]

    kwarm = _pick_kwarm(a)
    lc = HALF + kwarm
    nc = _get_module(kwarm)

    wdT = np.ascontiguousarray(W_down.T).astype(np.float16)
    wuT = np.ascontiguousarray(W_up.T).astype(np.float16)
    decay = np.ascontiguousarray(a.T.reshape(NME, P, NL))          # [t, p, l]
    scale_r = np.ascontiguousarray(scale.reshape(NME, P, 1))

    in_maps = []
    for c in range(N_CORES):
        b, h = divmod(c, 2)
        xt = np.zeros((lc, D), dtype=np.float32)
        lstart = h * HALF - kwarm
        src_lo = max(0, lstart)
        xt[src_lo - lstart :, :] = x[b, src_lo : h * HALF + HALF, :]
        xT = np.ascontiguousarray(xt.T).astype(np.float16)          # [D, lc]
        in_maps.append(
            {"xT": xT, "wdT": wdT, "wuT": wuT, "decay": decay, "scale": scale_r}
        )

    res = run_bass_kernel_spmd(nc, in_maps, core_ids=list(range(N_CORES)))
    LAST_RESULTS = res

    out = np.empty((B, L, D), dtype=np.float32)
    for c in range(N_CORES):
        b, h = divmod(c, 2)
        out[b, h * HALF : (h + 1) * HALF, :] = res.results[c]["outT"].T
    return out
